# revision 1
# baseline (speedup 1.0000x reference)
"""Trainium2 Bass kernel for nn_Detector (region-sum pooling + softmax).

The reference computes softmax(x.reshape(B, H*W) @ filt) where filt is a
fixed 0/1 mask selecting 10 disjoint 113x113 rectangular regions of the
1024x1024 image.  The dense GEMM is really a sparse pooling: out[b, k]
is the sum of x[b] over region k.  Only ~12% of x is ever needed, so we
DMA exactly the 10 regions per image instead of streaming all 512 MB.

Distribution: data-parallel over batch.  8 NeuronCores x 16 images each.
Per core and region: one DMA brings [16 batches x 113 rows x 113 cols]
into SBUF as [113 rows (partitions), 16 batches, 113 cols]; VectorE
reduces the col axis; a ones-vector matmul on TensorE reduces the row
(partition) axis, landing y[b, k] in PSUM with batches on partitions;
ScalarE/VectorE do the 10-wide softmax; one tiny DMA stores [16, 10].
"""

import numpy as np

import concourse.bass as bass
import concourse.tile as tile
from concourse import bacc, mybir
from concourse.bass_utils import run_bass_kernel_spmd

# Problem geometry — fixed by the reference's _build_filter(1024, 1024).
B, H, W = 128, 1024, 1024
S = 113  # min(1024 // 9, 1024 // 7)
REGIONS = [(2, 1), (2, 4), (2, 7), (4, 1), (4, 3), (4, 5), (4, 7), (6, 1), (6, 4), (6, 7)]
K = len(REGIONS)
N_CORES = 8
BPC = B // N_CORES  # images per core
F32 = mybir.dt.float32


def build_nc():
    nc = bacc.Bacc("TRN2", target_bir_lowering=False, debug=False)
    x = nc.declare_dram_parameter("x", [BPC, H, W], F32, isOutput=False)
    out = nc.declare_dram_parameter("out", [BPC, K], F32, isOutput=True)

    with tile.TileContext(nc) as tc:
        with (
            tc.tile_pool(name="reg", bufs=4) as rpool,
            tc.tile_pool(name="small", bufs=1) as spool,
            tc.tile_pool(name="psum", bufs=1, space=bass.MemorySpace.PSUM) as ppool,
        ):
            ones = spool.tile([S, 1], F32)
            nc.vector.memset(ones[:], 1.0)

            # Row partials: partition = region row, free = (region, batch).
            partials = spool.tile([S, K, BPC], F32)
            for k, (rb, cb) in enumerate(REGIONS):
                r0, c0 = rb * S, cb * S
                rt = rpool.tile([S, BPC, S], F32, tag="rt")
                nc.sync.dma_start(
                    out=rt[:],
                    in_=x[:, r0:r0 + S, c0:c0 + S].rearrange("b r c -> r b c"),
                )
                nc.vector.reduce_sum(
                    out=partials[:, k, :], in_=rt[:], axis=mybir.AxisListType.X
                )

            # Partition-axis reduce: y[b, k] = sum_r partials[r, k, b].
            py = ppool.tile([BPC, K], F32)
            for k in range(K):
                nc.tensor.matmul(
                    py[:, k:k + 1], partials[:, k, :], ones[:], start=True, stop=True
                )

            # Softmax over the 10 detectors, batches on partitions.
            m = spool.tile([BPC, 1], F32)
            nc.vector.reduce_max(m[:], py[:], axis=mybir.AxisListType.X)
            negm = spool.tile([BPC, 1], F32)
            nc.vector.tensor_scalar_mul(negm[:], m[:], -1.0)
            e = spool.tile([BPC, K], F32)
            ssum = spool.tile([BPC, 1], F32)
            nc.scalar.activation(
                e[:], py[:], mybir.ActivationFunctionType.Exp,
                bias=negm[:], accum_out=ssum[:],
            )
            rcp = spool.tile([BPC, 1], F32)
            nc.vector.reciprocal(rcp[:], ssum[:])
            o = spool.tile([BPC, K], F32)
            nc.scalar.mul(o[:], e[:], rcp[:])
            nc.sync.dma_start(out=out[:], in_=o[:])

    nc.compile()
    return nc


_NC = None


def get_nc():
    global _NC
    if _NC is None:
        _NC = build_nc()
    return _NC


def kernel(x, filt=None, **_unused):
    nc = get_nc()
    x = np.ascontiguousarray(np.asarray(x, dtype=np.float32))
    assert x.shape == (B, H, W), x.shape
    in_maps = [{"x": x[i * BPC:(i + 1) * BPC]} for i in range(N_CORES)]
    res = run_bass_kernel_spmd(nc, in_maps, list(range(N_CORES)))
    return np.concatenate([r["out"] for r in res.results], axis=0)


# revision 3
# speedup vs baseline: 1.0000x; 1.0000x over previous
"""Trainium2 Bass kernel for nn_Detector (region-sum pooling + softmax).

The reference computes softmax(x.reshape(B, H*W) @ filt) where filt is a
fixed 0/1 mask selecting 10 disjoint 113x113 rectangular regions of the
1024x1024 image.  The dense GEMM is really a sparse pooling: out[b, k]
is the sum of x[b] over region k.  Only ~12% of x is ever needed, so we
DMA exactly the 10 regions per image instead of streaming all 512 MB.

Distribution: data-parallel over batch.  8 NeuronCores x 16 images each.
Per core and region: one DMA brings [16 batches x 113 rows x 113 cols]
into SBUF as [113 rows (partitions), 16 batches, 113 cols]; VectorE
reduces the col axis; a ones-vector matmul on TensorE reduces the row
(partition) axis, landing y[b, k] in PSUM with batches on partitions;
ScalarE/VectorE do the 10-wide softmax; one tiny DMA stores [16, 10].
"""

import numpy as np

import concourse.bass as bass
import concourse.tile as tile
from concourse import bacc, mybir
from concourse.bass_utils import run_bass_kernel_spmd

# Problem geometry — fixed by the reference's _build_filter(1024, 1024).
B, H, W = 128, 1024, 1024
S = 113  # min(1024 // 9, 1024 // 7)
REGIONS = [(2, 1), (2, 4), (2, 7), (4, 1), (4, 3), (4, 5), (4, 7), (6, 1), (6, 4), (6, 7)]
K = len(REGIONS)
N_CORES = 8
BPC = B // N_CORES  # images per core
F32 = mybir.dt.float32


def build_nc():
    nc = bacc.Bacc("TRN2", target_bir_lowering=False, debug=False)
    x = nc.declare_dram_parameter("x", [BPC, H, W], F32, isOutput=False)
    out = nc.declare_dram_parameter("out", [BPC, K], F32, isOutput=True)

    with tile.TileContext(nc) as tc:
        with (
            tc.tile_pool(name="reg", bufs=4) as rpool,
            tc.tile_pool(name="small", bufs=1) as spool,
            tc.tile_pool(name="psum", bufs=1, space=bass.MemorySpace.PSUM) as ppool,
        ):
            ones = spool.tile([S, 1], F32)
            nc.vector.memset(ones[:], 1.0)

            # Row partials: partition = region row, free = (region, batch).
            partials = spool.tile([S, K, BPC], F32)
            for k, (rb, cb) in enumerate(REGIONS):
                r0, c0 = rb * S, cb * S
                rt = rpool.tile([S, BPC, S], F32, tag="rt")
                # SWDGE (gpsimd): Q7 sprays descriptors across all 16 SDMA
                # engines.  The HWDGE (sync) dynamic ring funnels every
                # descriptor through ONE engine (~19 GB/s measured).
                nc.sync.dma_start(
                    out=rt[:],
                    in_=x[:, r0:r0 + S, c0:c0 + S].rearrange("b r c -> r b c"),
                )
                nc.vector.reduce_sum(
                    out=partials[:, k, :], in_=rt[:], axis=mybir.AxisListType.X
                )

            # Partition-axis reduce: y[b, k] = sum_r partials[r, k, b].
            py = ppool.tile([BPC, K], F32)
            for k in range(K):
                nc.tensor.matmul(
                    py[:, k:k + 1], partials[:, k, :], ones[:], start=True, stop=True
                )

            # Softmax over the 10 detectors, batches on partitions.
            m = spool.tile([BPC, 1], F32)
            nc.vector.reduce_max(m[:], py[:], axis=mybir.AxisListType.X)
            negm = spool.tile([BPC, 1], F32)
            nc.vector.tensor_scalar_mul(negm[:], m[:], -1.0)
            e = spool.tile([BPC, K], F32)
            ssum = spool.tile([BPC, 1], F32)
            nc.scalar.activation(
                e[:], py[:], mybir.ActivationFunctionType.Exp,
                bias=negm[:], accum_out=ssum[:],
            )
            rcp = spool.tile([BPC, 1], F32)
            nc.vector.reciprocal(rcp[:], ssum[:])
            o = spool.tile([BPC, K], F32)
            nc.scalar.mul(o[:], e[:], rcp[:])
            nc.sync.dma_start(out=out[:], in_=o[:])

    nc.compile()
    return nc


_NC = None


def get_nc():
    global _NC
    if _NC is None:
        _NC = build_nc()
    return _NC


def kernel(x, filt=None, **_unused):
    nc = get_nc()
    x = np.ascontiguousarray(np.asarray(x, dtype=np.float32))
    assert x.shape == (B, H, W), x.shape
    in_maps = [{"x": x[i * BPC:(i + 1) * BPC]} for i in range(N_CORES)]
    res = run_bass_kernel_spmd(nc, in_maps, list(range(N_CORES)))
    return np.concatenate([r["out"] for r in res.results], axis=0)


# revision 9
# speedup vs baseline: 6.9152x; 6.9150x over previous
"""Trainium2 Bass kernel for nn_Detector (region-sum pooling + softmax).

The reference computes softmax(x.reshape(B, H*W) @ filt) where filt is a
fixed 0/1 mask selecting 10 disjoint 113x113 rectangular regions of the
1024x1024 image.  The dense GEMM is really a sparse pooling: out[b, k]
is the sum of x[b] over region k.  Only ~12% of x is ever needed, so we
DMA exactly the 10 regions per image instead of streaming all 512 MB.

Distribution: data-parallel over batch, 8 NeuronCores x 16 images each.

Per core and region, one SWDGE (gpsimd) DMA loads rows r0..r0+111 of the
region for all 16 images: DRAM side is the plain monotonic 3D slice
x[:, r0:r0+112, c0:c0+113]; SBUF side is [128, 14, 113] with partition =
(batch, row-octet).  Both sides enumerate elements in the same order, so
no AP rearrange is needed (SWDGE crashes on non-monotonic or 4D APs, and
HWDGE dynamic queues execute on a single SDMA engine at ~19 GB/s, so
this layout is the only fast path).  The 113th row goes via a small
HWDGE DMA on the sync queue in parallel.

Compute: VectorE reduces (rows-in-octet, cols) per region -> [128, 1]
partials; one TensorE matmul with a 0/1 block-indicator [128, 16]
contracts the 8 octets per batch -> PSUM [16, 10]; VectorE adds the
remainder-row partials; ScalarE does the numerically-stable softmax.
"""

import numpy as np

import concourse.bass as bass
import concourse.tile as tile
from concourse import bacc, mybir
from concourse.bass_utils import run_bass_kernel_spmd

# Problem geometry — fixed by the reference's _build_filter(1024, 1024).
B, H, W = 128, 1024, 1024
S = 113  # min(1024 // 9, 1024 // 7)
REGIONS = [(2, 1), (2, 4), (2, 7), (4, 1), (4, 3), (4, 5), (4, 7), (6, 1), (6, 4), (6, 7)]
K = len(REGIONS)
N_CORES = 8
BPC = B // N_CORES  # images per core
F32 = mybir.dt.float32
OCT, GR = 8, 14  # 112 of the 113 region rows = 8 octets x 14 rows


def build_nc():
    nc = bacc.Bacc("TRN2", target_bir_lowering=False, debug=False)
    x = nc.declare_dram_parameter("x", [BPC, H, W], F32, isOutput=False)
    blk_d = nc.declare_dram_parameter("blk", [128, BPC], F32, isOutput=False)
    out = nc.declare_dram_parameter("out", [BPC, K], F32, isOutput=True)

    with tile.TileContext(nc) as tc:
        with (
            tc.tile_pool(name="reg", bufs=4) as rpool,
            tc.tile_pool(name="small", bufs=1) as spool,
            tc.tile_pool(name="psum", bufs=1, space=bass.MemorySpace.PSUM) as ppool,
        ):
            # Block indicator: blk[p, b] = 1 iff p // 8 == b (sums octets
            # per batch in the matmul below).  Host-provided — engine
            # memsets can only start at partition 0/32/64/96.
            blk = spool.tile([128, BPC], F32)
            nc.sync.dma_start(out=blk[:], in_=blk_d[:])

            rem = spool.tile([BPC, K, S], F32)
            mpart = spool.tile([128, K], F32)
            for k, (rb, cb) in enumerate(REGIONS):
                r0, c0 = rb * S, cb * S
                mt = rpool.tile([128, GR, S], F32, tag="mt")
                nc.gpsimd.dma_start(
                    out=mt[:], in_=x[:, r0:r0 + OCT * GR, c0:c0 + S]
                )
                nc.sync.dma_start(
                    out=rem[:, k, :], in_=x[:, r0 + OCT * GR, c0:c0 + S]
                )
                nc.vector.reduce_sum(
                    out=mpart[:, k:k + 1], in_=mt[:], axis=mybir.AxisListType.XY
                )

            rpart = spool.tile([BPC, K], F32)
            nc.vector.reduce_sum(out=rpart[:], in_=rem[:], axis=mybir.AxisListType.X)

            py = ppool.tile([BPC, K], F32)
            nc.tensor.matmul(py[:], blk[:], mpart[:], start=True, stop=True)

            ys = spool.tile([BPC, K], F32)
            nc.vector.tensor_add(ys[:], py[:], rpart[:])

            # Softmax over the 10 detectors, batches on partitions.
            m = spool.tile([BPC, 1], F32)
            nc.vector.reduce_max(m[:], ys[:], axis=mybir.AxisListType.X)
            negm = spool.tile([BPC, 1], F32)
            nc.vector.tensor_scalar_mul(negm[:], m[:], -1.0)
            e = spool.tile([BPC, K], F32)
            ssum = spool.tile([BPC, 1], F32)
            nc.scalar.activation(
                e[:], ys[:], mybir.ActivationFunctionType.Exp,
                bias=negm[:], accum_out=ssum[:],
            )
            rcp = spool.tile([BPC, 1], F32)
            nc.vector.reciprocal(rcp[:], ssum[:])
            o = spool.tile([BPC, K], F32)
            nc.scalar.mul(o[:], e[:], rcp[:])
            nc.sync.dma_start(out=out[:], in_=o[:])

    nc.compile()
    return nc


_NC = None


def get_nc():
    global _NC
    if _NC is None:
        _NC = build_nc()
    return _NC


def kernel(x, filt=None, **_unused):
    nc = get_nc()
    x = np.ascontiguousarray(np.asarray(x, dtype=np.float32))
    assert x.shape == (B, H, W), x.shape
    blk = np.repeat(np.eye(BPC, dtype=np.float32), OCT, axis=0)
    in_maps = [
        {"x": x[i * BPC:(i + 1) * BPC], "blk": blk} for i in range(N_CORES)
    ]
    res = run_bass_kernel_spmd(nc, in_maps, list(range(N_CORES)))
    return np.concatenate([r["out"] for r in res.results], axis=0)
